# revision 25
# baseline (speedup 1.0000x reference)
"""Trainium2 Bass kernel for nn_ContrastiveCosineLoss.

loss = mean_{i<j} (cos(f_i,f_j) - cos(r_i,r_j))^2 over N=2048 rows.

Math: with Fn/Rn the row-normalized embeddings and
  Gf = Fn^T Fn  [1024,1024],  Gr = Rn^T Rn  [128,128],  X = Fn^T Rn  [1024,128]
  loss = (||Gf||_F^2 - 2||X||_F^2 + ||Gr||_F^2) / (2M),  M = N(N-1)/2
(diagonal diffs vanish exactly: cos(i,i)=1 on both sides).

All matmul data is fp8_e4m3 (cast on host; final loss error ~1e-3 vs the fp32
reference). fp8 enables DoubleRow matmuls and 4x less DMA than fp32.

Sharding (8 cores, SPMD single program; per-core differences only via host
packing): core c gets F8 column-rolled by c*128 (its local feature strip 0 =
global strip c) AND chunk(row)-rolled by 2c*128; R8 is chunk-rolled the same
way. Per core:
  - Gf row-strip c: lhsT = la = strip0 * (SA/nf^2), moving = raw F chunks.
  - X row-strip c: same lhsT, moving = rx = R8 * (nf/(4*nr)).
  - Gr: contraction-sharded via the first two (rolled = own) R chunks; raw
    partial Gram out, host-reduced before squaring. The chunk roll makes
    slots 0:2 a distinct 256-row slice per core covering all rows.
  - Norms nf^2/nr^2 on device, pipelined per 2-chunk DMA arrival across
    ACT/DVE (R path on GPSIMD); per-group (4 chunks) recip -> la -> Gf
    matmuls -> gg -> rx -> X matmuls: no global norm barrier anywhere.

All input DMAs ride the sync HWDGE ring (descriptor-gen runs on the issuing
engine's sequencer, so putting them on scalar/gpsimd stalls ACT compute or
faults SWDGE): front-half F in 2-chunk (256 KB) grain so squares start as
each pair lands, rap interleaved third, back-half F as one 1 MB transfer.
F squares split ACT 9 / DVE 7 (GPSIMD's tensor_tensor is ~2.5x slower and
its reduce would tax DVE; it does the R squares, rx, and lb instead).
A run of garbage matmuls keeps the PE busy from t=0 toward the first real
matmul to ramp the p-state clock.

fp8 operands carry power-of-2 compensation scales (SA/SX/SB) to sit in
e4m3's normal range; the host divides them back out.
"""

import numpy as np
import ml_dtypes

N_ROWS = 2048
KF = 1024
KR = 128
P = 128
NCH = N_ROWS // P          # 16 contraction chunks of 128 rows
GRP = 4                    # chunks per scale/matmul group
NG = NCH // GRP
M_PAIRS = N_ROWS * (N_ROWS - 1) // 2
EPS2 = 1e-16               # max(norm,1e-8)^2 clamp, applied to norm^2
SA = 1024.0                # la = F8 * (SA/nf^2)
SX = 256.0                 # net X element scale: (SA/nf^2)*(nf/(4 nr)) = SX/(nf nr)
SB = 128.0                 # lb = R8 * (SB/nr^2)

TRACE = False              # test.py flips this (needs the axon NTFF shim)
LAST_EXEC_NS = None

_CACHED = {}

F8NP = ml_dtypes.float8_e4m3

def _build():
    import concourse.bacc as bacc
    import concourse.mybir as mybir
    from concourse.tile import TileContext
    from concourse.alu_op_type import AluOpType

    F32 = mybir.dt.float32
    F8 = mybir.dt.float8e4
    BF16 = mybir.dt.bfloat16
    ACTF = mybir.ActivationFunctionType
    AX = mybir.AxisListType
    DR = mybir.MatmulPerfMode.DoubleRow

    nc = bacc.Bacc("TRN2", num_devices=8)
    fmv = nc.dram_tensor("fmv", [P, NCH * KF], F8, kind="ExternalInput")
    rap = nc.dram_tensor("rap", [P, NCH * KR], F8, kind="ExternalInput")
    out_d = nc.dram_tensor("out_d", [P, KR + 3], F32, kind="ExternalOutput")

    with TileContext(nc) as tc:
        with (
            tc.tile_pool(name="big_p", bufs=1) as big_p,
            tc.tile_pool(name="scr_p", bufs=2) as scr_p,
            tc.tile_pool(name="nrm_p", bufs=1) as nrm_p,
            tc.tile_pool(name="psum", bufs=1, space="PSUM") as psum_p,
        ):
            f_all = big_p.tile([P, NCH, KF], F8)
            ra_all = big_p.tile([P, NCH, KR], F8)
            rx_all = big_p.tile([P, NCH, KR], F8)
            la_all = big_p.tile([P, NCH, P], F8, tag="la")

            # --- input DMAs: all on the sync ring, so descriptor-gen
            # stays off the compute engines' sequencers (on scalar it
            # blocks ACT dispatch; gpsimd SWDGE faulted). Front-half F in
            # 2-chunk grain for pipelined arrival, rap third, back half
            # as one 1MB transfer.
            def fpair(t):
                nc.sync.dma_start(
                    f_all[:, 2 * t:2 * t + 2, :],
                    fmv[:, 2 * t * KF:(2 * t + 2) * KF].rearrange(
                        "p (k j) -> p k j", j=KF),
                )

            fpair(0)
            fpair(1)
            nc.sync.dma_start(
                ra_all[:], rap[:].rearrange("p (k j) -> p k j", j=KR)
            )
            fpair(2)
            fpair(3)
            nc.sync.dma_start(
                f_all[:, 8:12, :],
                fmv[:, 8 * KF:12 * KF].rearrange("p (k j) -> p k j", j=KF),
            )
            nc.sync.dma_start(
                f_all[:, 12:16, :],
                fmv[:, 12 * KF:].rearrange("p (k j) -> p k j", j=KF),
            )

            # --- PE p-state warmup: keep PE busy until the first real matmul
            wsrc = nrm_p.tile([P, 2, 512], F8)
            nc.gpsimd.memset(wsrc[:], 1)
            psA0 = psum_p.tile([P, 512], F32, tag="a0", name="psA0")
            psA1 = psum_p.tile([P, 512], F32, tag="a1", name="psA1")
            psX = psum_p.tile([P, KR], F32, tag="x", name="psX")
            psB = psum_p.tile([P, KR], F32, tag="b", name="psB")
            psW = psum_p.tile([P, 512], F32, tag="w", name="psW")
            for i in range(26):
                nc.tensor.matmul(
                    psW[:], lhsT=wsrc[:, :, 0:P], rhs=wsrc[:],
                    start=True, stop=True, perf_mode=DR,
                )

            # --- ACT table warmup: Sqrt first -> sqrt_and_others set, which
            # also contains Square, so ideally a single table load.
            warm = nrm_p.tile([P, 1], F32)
            nc.vector.memset(warm[:], 1.0)
            warm2 = nrm_p.tile([P, 1], F32)
            nc.scalar.activation(warm2[:], warm[:], ACTF.Sqrt)
            nc.scalar.activation(warm2[:], warm[:], ACTF.Square)

            nf2 = nrm_p.tile([P, NCH], F32, tag="nf2")
            nr2 = nrm_p.tile([P, NCH], F32, tag="nr2")
            af_all = nrm_p.tile([P, NCH], F32, tag="af")
            rr_all = nrm_p.tile([P, NCH], F32, tag="rr")
            vv_all = nrm_p.tile([P, NCH], F32, tag="vv")
            gg_all = nrm_p.tile([P, NCH], F32, tag="gg")
            rsq = nrm_p.tile([P, NCH, KR], BF16, tag="rsq")
            out_sb = nrm_p.tile([P, KR + 3], F32, tag="osb")

            ISA = float(1.0 / SA)
            RSA = float(1.0 / np.sqrt(SA))

            def sq_act(k):
                s = scr_p.tile([P, KF], BF16, tag="sA", name=f"sA{k}")
                nc.scalar.activation(
                    s[:], f_all[:, k, :], ACTF.Square, scale=RSA,
                    accum_out=nf2[:, k:k + 1],
                )

            def sq_dve(k):
                s = scr_p.tile([P, KF], BF16, tag="sD", name=f"sD{k}")
                nc.vector.scalar_tensor_tensor(
                    s[:], f_all[:, k, :], ISA, f_all[:, k, :],
                    AluOpType.mult, AluOpType.mult,
                    accum_out=nf2[:, k:k + 1],
                )

            # R squares: two 1024-wide GPSIMD passes; segmented DVE reduces.
            def r_square(half):
                sl = slice(8 * half, 8 * (half + 1))
                nc.gpsimd.tensor_tensor(
                    rsq[:, sl, :].rearrange("p k j -> p (k j)"),
                    ra_all[:, sl, :].rearrange("p k j -> p (k j)"),
                    ra_all[:, sl, :].rearrange("p k j -> p (k j)"),
                    AluOpType.mult,
                )

            def r_reduce(half):
                sl = slice(8 * half, 8 * (half + 1))
                nc.vector.reduce_sum(nr2[:, sl], rsq[:, sl, :], axis=AX.X)
                tr = nrm_p.tile([P, 8], F32, name=f"tr{half}")
                nc.vector.tensor_scalar_max(tr[:], nr2[:, sl], EPS2)
                nc.vector.reciprocal(rr_all[:, sl], tr[:])

            def recip_la(g):
                sl = slice(GRP * g, GRP * (g + 1))
                nc.vector.reciprocal(af_all[:, sl], nf2[:, sl])
                nc.vector.tensor_tensor(
                    la_all[:, sl, :], f_all[:, sl, 0:P],
                    af_all[:, sl, None].broadcast_to([P, GRP, P]),
                    AluOpType.mult,
                )
                nc.vector.tensor_tensor(
                    vv_all[:, sl], nf2[:, sl], rr_all[:, sl], AluOpType.mult
                )

            def gf_mms(g):
                for t in (2 * g, 2 * g + 1):
                    st = dict(start=(t == 0), stop=(t == NCH // 2 - 1))
                    ksl = slice(2 * t, 2 * t + 2)
                    nc.tensor.matmul(
                        psA0[:], lhsT=la_all[:, ksl, :],
                        rhs=f_all[:, ksl, 0:512], perf_mode=DR, **st
                    )
                    nc.tensor.matmul(
                        psA1[:], lhsT=la_all[:, ksl, :],
                        rhs=f_all[:, ksl, 512:KF], perf_mode=DR, **st
                    )

            def gg_act(g):
                sl = slice(GRP * g, GRP * (g + 1))
                # gg = sqrt(vv * SA/16) = nf/(4 nr); scale applies inside.
                nc.scalar.activation(
                    gg_all[:, sl], vv_all[:, sl], ACTF.Sqrt, scale=SA / 16.0
                )

            def rx_gps(g):
                sl = slice(GRP * g, GRP * (g + 1))
                nc.gpsimd.tensor_tensor(
                    rx_all[:, sl, :], ra_all[:, sl, :],
                    gg_all[:, sl, None].broadcast_to([P, GRP, KR]),
                    AluOpType.mult,
                )

            def x_mms(g):
                for t in (2 * g, 2 * g + 1):
                    st = dict(start=(t == 0), stop=(t == NCH // 2 - 1))
                    ksl = slice(2 * t, 2 * t + 2)
                    nc.tensor.matmul(
                        psX[:], lhsT=la_all[:, ksl, :],
                        rhs=rx_all[:, ksl, :], perf_mode=DR, **st
                    )

            # --- emission: squares in arrival order; ACT 9 / DVE 7;
            # GPSIMD: R squares, rx, lb.
            r_square(0)                      # GPS (rap lands third)
            sq_act(0)
            sq_dve(1)
            sq_act(2)
            sq_dve(3)
            r_reduce(0)                      # DVE

            # Gr: lb from the first two (own) R chunks; raw rhs.
            nb = nrm_p.tile([P, 2], F32, tag="nb")
            nc.vector.tensor_scalar(
                nb[:], nr2[:, 0:2], EPS2, float(1.0 / SB),
                AluOpType.max, AluOpType.mult,
            )
            bb = nrm_p.tile([P, 2], F32, tag="bb")
            nc.vector.reciprocal(bb[:], nb[:])
            lb = nrm_p.tile([P, 2, KR], F8, tag="lb")
            nc.gpsimd.tensor_tensor(
                lb[:], ra_all[:, 0:2, :],
                bb[:, :, None].broadcast_to([P, 2, KR]), AluOpType.mult,
            )

            r_square(1)                      # GPS
            sq_act(4)
            sq_dve(5)
            recip_la(0)
            gf_mms(0)
            nc.tensor.matmul(
                psB[:], lhsT=lb[:], rhs=ra_all[:, 0:2, :],
                start=True, stop=True, perf_mode=DR,
            )
            sq_act(6)
            sq_dve(7)
            r_reduce(1)                      # DVE
            gg_act(0)
            rx_gps(0)
            sq_act(8)
            sq_dve(9)
            recip_la(1)
            gf_mms(1)
            x_mms(0)
            gg_act(1)
            rx_gps(1)
            sq_act(10)
            sq_dve(11)
            recip_la(2)
            gf_mms(2)
            x_mms(1)
            gg_act(2)
            rx_gps(2)
            sq_act(12)
            sq_dve(13)
            x_mms(2)
            sq_act(14)
            sq_act(15)
            # rx2-anchored garbage matmuls: bridge the PE gap between X2
            # and GfG3 so the G3 tail runs at the max p-state clock. The
            # rx dependency stops the scheduler from hoisting them into
            # the warmup, and they sit after X2 in the PE queue.
            for i in range(10):
                nc.tensor.matmul(
                    psW[:], lhsT=rx_all[:, 8:10, :], rhs=wsrc[:],
                    start=True, stop=True, perf_mode=DR,
                )
            recip_la(3)
            gf_mms(3)
            gg_act(3)
            rx_gps(3)
            x_mms(3)

            # --- epilogue: Frobenius partials + Gr partial -> one DMA ---
            # DVE can't read PSUM twice in one op, so psA1^2 goes
            # ACT-square -> bf16 scratch -> DVE reduce.
            sE1 = scr_p.tile([P, 512], BF16, tag="sE1")
            nc.scalar.activation(sE1[:], psA1[0:P, :], ACTF.Square)
            sE0 = scr_p.tile([P, 512], F32, tag="sE0")
            nc.scalar.activation(
                sE0[:], psA0[0:P, :], ACTF.Square,
                accum_out=out_sb[:, KR:KR + 1],
            )
            nc.vector.reduce_sum(
                out_sb[:, KR + 1:KR + 2], sE1[:], axis=AX.X
            )
            nc.vector.tensor_copy(out_sb[:, 0:KR], psB[0:P, :])
            sX1 = scr_p.tile([P, KR], F32, tag="sX1")
            nc.vector.tensor_copy(sX1[:], psX[0:P, :])
            sX2 = scr_p.tile([P, KR], F32, tag="sX2")
            nc.vector.scalar_tensor_tensor(
                sX2[:], sX1[:], 1.0, sX1[:],
                AluOpType.mult, AluOpType.mult,
                accum_out=out_sb[:, KR + 2:KR + 3],
            )
            nc.sync.dma_start(out_d[:], out_sb[:])

    nc.finalize()
    return nc


def _pack_rolled(a, nch, roll):
    # [nch*128, w] row-chunked -> chunk-rolled [128, nch*w] SBUF-native
    w = a.shape[1]
    ch = a.reshape(nch, P, w)
    if roll:
        ch = np.roll(ch, -roll, axis=0)
    return np.ascontiguousarray(ch.transpose(1, 0, 2).reshape(P, nch * w))


def kernel(reduced_embeddings: np.ndarray, full_embeddings: np.ndarray) -> np.ndarray:
    global LAST_EXEC_NS
    from concourse.bass_utils import run_bass_kernel_spmd

    F8 = full_embeddings.astype(F8NP)
    R8 = reduced_embeddings.astype(F8NP)

    if "nc" not in _CACHED:
        _CACHED["nc"] = _build()
    nc = _CACHED["nc"]

    in_maps = []
    for c in range(8):
        fa = np.roll(F8, -(c * P), axis=1)
        in_maps.append({
            "fmv": _pack_rolled(fa, NCH, 2 * c),
            "rap": _pack_rolled(R8, NCH, 2 * c),
        })

    kw = {}
    if TRACE:
        kw = dict(trace=True, trace_cores=[0])
    res = run_bass_kernel_spmd(nc, in_maps, core_ids=list(range(8)), **kw)
    LAST_EXEC_NS = res.exec_time_ns

    s_gf = 0.0
    s_x = 0.0
    gr = np.zeros((P, KR), dtype=np.float64)
    for c in range(8):
        o = res.results[c]["out_d"].astype(np.float64)
        gr += o[:, 0:KR]
        s_gf += float(o[:, KR].sum() + o[:, KR + 1].sum())
        s_x += float(o[:, KR + 2].sum())
    s_gf /= SA * SA
    s_x /= SX * SX
    gr /= SB
    s_gr = float((gr * gr).sum())
    loss = (s_gf - 2.0 * s_x + s_gr) / (2.0 * M_PAIRS)
    return np.float32(loss)


# revision 26
# speedup vs baseline: 1.0397x; 1.0397x over previous
"""Trainium2 Bass kernel for nn_ContrastiveCosineLoss.

loss = mean_{i<j} (cos(f_i,f_j) - cos(r_i,r_j))^2 over N=2048 rows.

Math: with Fn/Rn the row-normalized embeddings and
  Gf = Fn^T Fn  [1024,1024],  Gr = Rn^T Rn  [128,128],  X = Fn^T Rn  [1024,128]
  loss = (||Gf||_F^2 - 2||X||_F^2 + ||Gr||_F^2) / (2M),  M = N(N-1)/2
(diagonal diffs vanish exactly: cos(i,i)=1 on both sides).

All matmul data is fp8_e4m3 (cast on host; final loss error ~1e-3 vs the fp32
reference). fp8 enables DoubleRow matmuls and 4x less DMA than fp32.

Sharding (8 cores, SPMD single program; per-core differences only via host
packing): core c gets F8 column-rolled by c*128 (its local feature strip 0 =
global strip c) AND chunk(row)-rolled by 2c*128; R8 is chunk-rolled the same
way. Per core:
  - Gf row-strip c: lhsT = la = strip0 * (SA/nf^2), moving = raw F chunks.
  - X row-strip c: same lhsT, moving = rx = R8 * (nf/(4*nr)).
  - Gr: contraction-sharded via the first two (rolled = own) R chunks; raw
    partial Gram out, host-reduced before squaring. The chunk roll makes
    slots 0:2 a distinct 256-row slice per core covering all rows.
  - Norms nf^2/nr^2 on device, pipelined per 2-chunk DMA arrival across
    ACT/DVE (R path on GPSIMD); per-group (4 chunks) recip -> la -> Gf
    matmuls -> gg -> rx -> X matmuls: no global norm barrier anywhere.

All input DMAs ride the sync HWDGE ring (descriptor-gen runs on the issuing
engine's sequencer, so putting them on scalar/gpsimd stalls ACT compute or
faults SWDGE): front-half F in 2-chunk (256 KB) grain so squares start as
each pair lands, rap interleaved third, back-half F as two 512 KB
transfers so completion receipts pipeline.
F squares split ACT 9 / DVE 7 (GPSIMD's tensor_tensor is ~2.5x slower and
its reduce would tax DVE; it does the R squares, rx, and lb instead).
A run of garbage matmuls keeps the PE busy from t=0 toward the first real
matmul to ramp the p-state clock.

fp8 operands carry power-of-2 compensation scales (SA/SX/SB) to sit in
e4m3's normal range; the host divides them back out.
"""

import numpy as np
import ml_dtypes

N_ROWS = 2048
KF = 1024
KR = 128
P = 128
NCH = N_ROWS // P          # 16 contraction chunks of 128 rows
GRP = 4                    # chunks per scale/matmul group
NG = NCH // GRP
M_PAIRS = N_ROWS * (N_ROWS - 1) // 2
EPS2 = 1e-16               # max(norm,1e-8)^2 clamp, applied to norm^2
SA = 1024.0                # la = F8 * (SA/nf^2)
SX = 256.0                 # net X element scale: (SA/nf^2)*(nf/(4 nr)) = SX/(nf nr)
SB = 128.0                 # lb = R8 * (SB/nr^2)

TRACE = False              # test.py flips this (needs the axon NTFF shim)
LAST_EXEC_NS = None

_CACHED = {}

F8NP = ml_dtypes.float8_e4m3

def _build():
    import concourse.bacc as bacc
    import concourse.mybir as mybir
    from concourse.tile import TileContext
    from concourse.alu_op_type import AluOpType

    F32 = mybir.dt.float32
    F8 = mybir.dt.float8e4
    BF16 = mybir.dt.bfloat16
    ACTF = mybir.ActivationFunctionType
    AX = mybir.AxisListType
    DR = mybir.MatmulPerfMode.DoubleRow

    nc = bacc.Bacc("TRN2", num_devices=8)
    fmv = nc.dram_tensor("fmv", [P, NCH * KF], F8, kind="ExternalInput")
    rap = nc.dram_tensor("rap", [P, NCH * KR], F8, kind="ExternalInput")
    out_d = nc.dram_tensor("out_d", [P, KR + 3], F32, kind="ExternalOutput")

    with TileContext(nc) as tc:
        with (
            tc.tile_pool(name="big_p", bufs=1) as big_p,
            tc.tile_pool(name="scr_p", bufs=2) as scr_p,
            tc.tile_pool(name="nrm_p", bufs=1) as nrm_p,
            tc.tile_pool(name="psum", bufs=1, space="PSUM") as psum_p,
        ):
            f_all = big_p.tile([P, NCH, KF], F8)
            ra_all = big_p.tile([P, NCH, KR], F8)
            rx_all = big_p.tile([P, NCH, KR], F8)
            la_all = big_p.tile([P, NCH, P], F8, tag="la")

            # --- input DMAs: all on the sync ring, so descriptor-gen
            # stays off the compute engines' sequencers (on scalar it
            # blocks ACT dispatch; gpsimd SWDGE faulted). Front-half F in
            # 2-chunk grain for pipelined arrival, rap third, back half
            # as one 1MB transfer.
            def fpair(t):
                nc.sync.dma_start(
                    f_all[:, 2 * t:2 * t + 2, :],
                    fmv[:, 2 * t * KF:(2 * t + 2) * KF].rearrange(
                        "p (k j) -> p k j", j=KF),
                )

            fpair(0)
            fpair(1)
            nc.sync.dma_start(
                ra_all[:], rap[:].rearrange("p (k j) -> p k j", j=KR)
            )
            fpair(2)
            fpair(3)
            nc.sync.dma_start(
                f_all[:, 8:12, :],
                fmv[:, 8 * KF:12 * KF].rearrange("p (k j) -> p k j", j=KF),
            )
            nc.sync.dma_start(
                f_all[:, 12:16, :],
                fmv[:, 12 * KF:].rearrange("p (k j) -> p k j", j=KF),
            )

            # --- PE p-state warmup: keep PE busy until the first real matmul
            wsrc = nrm_p.tile([P, 2, 512], F8)
            nc.gpsimd.memset(wsrc[:], 1)
            psA0 = psum_p.tile([P, 512], F32, tag="a0", name="psA0")
            psA1 = psum_p.tile([P, 512], F32, tag="a1", name="psA1")
            psX = psum_p.tile([P, KR], F32, tag="x", name="psX")
            psB = psum_p.tile([P, KR], F32, tag="b", name="psB")
            psW = psum_p.tile([P, 512], F32, tag="w", name="psW")
            for i in range(26):
                nc.tensor.matmul(
                    psW[:], lhsT=wsrc[:, :, 0:P], rhs=wsrc[:],
                    start=True, stop=True, perf_mode=DR,
                )

            # --- ACT table warmup: Sqrt first -> sqrt_and_others set, which
            # also contains Square, so ideally a single table load.
            warm = nrm_p.tile([P, 1], F32)
            nc.vector.memset(warm[:], 1.0)
            warm2 = nrm_p.tile([P, 1], F32)
            nc.scalar.activation(warm2[:], warm[:], ACTF.Sqrt)
            nc.scalar.activation(warm2[:], warm[:], ACTF.Square)

            nf2 = nrm_p.tile([P, NCH], F32, tag="nf2")
            nr2 = nrm_p.tile([P, NCH], F32, tag="nr2")
            af_all = nrm_p.tile([P, NCH], F32, tag="af")
            rr_all = nrm_p.tile([P, NCH], F32, tag="rr")
            vv_all = nrm_p.tile([P, NCH], F32, tag="vv")
            gg_all = nrm_p.tile([P, NCH], F32, tag="gg")
            rsq = nrm_p.tile([P, NCH, KR], BF16, tag="rsq")
            out_sb = nrm_p.tile([P, KR + 3], F32, tag="osb")

            ISA = float(1.0 / SA)
            RSA = float(1.0 / np.sqrt(SA))

            def sq_act(k):
                s = scr_p.tile([P, KF], BF16, tag="sA", name=f"sA{k}")
                nc.scalar.activation(
                    s[:], f_all[:, k, :], ACTF.Square, scale=RSA,
                    accum_out=nf2[:, k:k + 1],
                )

            def sq_dve(k):
                s = scr_p.tile([P, KF], BF16, tag="sD", name=f"sD{k}")
                nc.vector.scalar_tensor_tensor(
                    s[:], f_all[:, k, :], ISA, f_all[:, k, :],
                    AluOpType.mult, AluOpType.mult,
                    accum_out=nf2[:, k:k + 1],
                )

            # R squares: two 1024-wide GPSIMD passes; segmented DVE reduces.
            def r_square(half):
                sl = slice(8 * half, 8 * (half + 1))
                nc.gpsimd.tensor_tensor(
                    rsq[:, sl, :].rearrange("p k j -> p (k j)"),
                    ra_all[:, sl, :].rearrange("p k j -> p (k j)"),
                    ra_all[:, sl, :].rearrange("p k j -> p (k j)"),
                    AluOpType.mult,
                )

            def r_reduce(half):
                sl = slice(8 * half, 8 * (half + 1))
                nc.vector.reduce_sum(nr2[:, sl], rsq[:, sl, :], axis=AX.X)
                tr = nrm_p.tile([P, 8], F32, name=f"tr{half}")
                nc.vector.tensor_scalar_max(tr[:], nr2[:, sl], EPS2)
                nc.vector.reciprocal(rr_all[:, sl], tr[:])

            def recip_la(g):
                sl = slice(GRP * g, GRP * (g + 1))
                nc.vector.reciprocal(af_all[:, sl], nf2[:, sl])
                nc.vector.tensor_tensor(
                    la_all[:, sl, :], f_all[:, sl, 0:P],
                    af_all[:, sl, None].broadcast_to([P, GRP, P]),
                    AluOpType.mult,
                )
                nc.vector.tensor_tensor(
                    vv_all[:, sl], nf2[:, sl], rr_all[:, sl], AluOpType.mult
                )

            def gf_mms(g):
                for t in (2 * g, 2 * g + 1):
                    st = dict(start=(t == 0), stop=(t == NCH // 2 - 1))
                    ksl = slice(2 * t, 2 * t + 2)
                    nc.tensor.matmul(
                        psA0[:], lhsT=la_all[:, ksl, :],
                        rhs=f_all[:, ksl, 0:512], perf_mode=DR, **st
                    )
                    nc.tensor.matmul(
                        psA1[:], lhsT=la_all[:, ksl, :],
                        rhs=f_all[:, ksl, 512:KF], perf_mode=DR, **st
                    )

            def gg_act(g):
                sl = slice(GRP * g, GRP * (g + 1))
                # gg = sqrt(vv * SA/16) = nf/(4 nr); scale applies inside.
                nc.scalar.activation(
                    gg_all[:, sl], vv_all[:, sl], ACTF.Sqrt, scale=SA / 16.0
                )

            def rx_gps(g):
                sl = slice(GRP * g, GRP * (g + 1))
                nc.gpsimd.tensor_tensor(
                    rx_all[:, sl, :], ra_all[:, sl, :],
                    gg_all[:, sl, None].broadcast_to([P, GRP, KR]),
                    AluOpType.mult,
                )

            def x_mms(g):
                for t in (2 * g, 2 * g + 1):
                    st = dict(start=(t == 0), stop=(t == NCH // 2 - 1))
                    ksl = slice(2 * t, 2 * t + 2)
                    nc.tensor.matmul(
                        psX[:], lhsT=la_all[:, ksl, :],
                        rhs=rx_all[:, ksl, :], perf_mode=DR, **st
                    )

            # --- emission: squares in arrival order; ACT 9 / DVE 7;
            # GPSIMD: R squares, rx, lb.
            r_square(0)                      # GPS (rap lands third)
            sq_act(0)
            sq_dve(1)
            sq_act(2)
            sq_dve(3)
            r_reduce(0)                      # DVE

            # Gr: lb from the first two (own) R chunks; raw rhs.
            nb = nrm_p.tile([P, 2], F32, tag="nb")
            nc.vector.tensor_scalar(
                nb[:], nr2[:, 0:2], EPS2, float(1.0 / SB),
                AluOpType.max, AluOpType.mult,
            )
            bb = nrm_p.tile([P, 2], F32, tag="bb")
            nc.vector.reciprocal(bb[:], nb[:])
            lb = nrm_p.tile([P, 2, KR], F8, tag="lb")
            nc.gpsimd.tensor_tensor(
                lb[:], ra_all[:, 0:2, :],
                bb[:, :, None].broadcast_to([P, 2, KR]), AluOpType.mult,
            )

            r_square(1)                      # GPS
            sq_act(4)
            sq_dve(5)
            recip_la(0)
            gf_mms(0)
            nc.tensor.matmul(
                psB[:], lhsT=lb[:], rhs=ra_all[:, 0:2, :],
                start=True, stop=True, perf_mode=DR,
            )
            sq_act(6)
            sq_dve(7)
            r_reduce(1)                      # DVE
            gg_act(0)
            rx_gps(0)
            sq_act(8)
            sq_dve(9)
            recip_la(1)
            gf_mms(1)
            x_mms(0)
            gg_act(1)
            rx_gps(1)
            sq_act(10)
            sq_dve(11)
            recip_la(2)
            gf_mms(2)
            x_mms(1)
            gg_act(2)
            rx_gps(2)
            sq_act(12)
            sq_dve(13)
            x_mms(2)
            sq_act(14)
            sq_act(15)
            recip_la(3)
            gf_mms(3)
            gg_act(3)
            rx_gps(3)
            x_mms(3)

            # --- epilogue: Frobenius partials + Gr partial -> one DMA ---
            # DVE can't read PSUM twice in one op, so psA1^2 goes
            # ACT-square -> bf16 scratch -> DVE reduce.
            sE1 = scr_p.tile([P, 512], BF16, tag="sE1")
            nc.scalar.activation(sE1[:], psA1[0:P, :], ACTF.Square)
            sE0 = scr_p.tile([P, 512], F32, tag="sE0")
            nc.scalar.activation(
                sE0[:], psA0[0:P, :], ACTF.Square,
                accum_out=out_sb[:, KR:KR + 1],
            )
            nc.vector.reduce_sum(
                out_sb[:, KR + 1:KR + 2], sE1[:], axis=AX.X
            )
            nc.vector.tensor_copy(out_sb[:, 0:KR], psB[0:P, :])
            sX1 = scr_p.tile([P, KR], F32, tag="sX1")
            nc.vector.tensor_copy(sX1[:], psX[0:P, :])
            sX2 = scr_p.tile([P, KR], F32, tag="sX2")
            nc.vector.scalar_tensor_tensor(
                sX2[:], sX1[:], 1.0, sX1[:],
                AluOpType.mult, AluOpType.mult,
                accum_out=out_sb[:, KR + 2:KR + 3],
            )
            nc.sync.dma_start(out_d[:], out_sb[:])

    nc.finalize()
    return nc


def _pack_rolled(a, nch, roll):
    # [nch*128, w] row-chunked -> chunk-rolled [128, nch*w] SBUF-native
    w = a.shape[1]
    ch = a.reshape(nch, P, w)
    if roll:
        ch = np.roll(ch, -roll, axis=0)
    return np.ascontiguousarray(ch.transpose(1, 0, 2).reshape(P, nch * w))


def kernel(reduced_embeddings: np.ndarray, full_embeddings: np.ndarray) -> np.ndarray:
    global LAST_EXEC_NS
    from concourse.bass_utils import run_bass_kernel_spmd

    F8 = full_embeddings.astype(F8NP)
    R8 = reduced_embeddings.astype(F8NP)

    if "nc" not in _CACHED:
        _CACHED["nc"] = _build()
    nc = _CACHED["nc"]

    in_maps = []
    for c in range(8):
        fa = np.roll(F8, -(c * P), axis=1)
        in_maps.append({
            "fmv": _pack_rolled(fa, NCH, 2 * c),
            "rap": _pack_rolled(R8, NCH, 2 * c),
        })

    kw = {}
    if TRACE:
        kw = dict(trace=True, trace_cores=[0])
    res = run_bass_kernel_spmd(nc, in_maps, core_ids=list(range(8)), **kw)
    LAST_EXEC_NS = res.exec_time_ns

    s_gf = 0.0
    s_x = 0.0
    gr = np.zeros((P, KR), dtype=np.float64)
    for c in range(8):
        o = res.results[c]["out_d"].astype(np.float64)
        gr += o[:, 0:KR]
        s_gf += float(o[:, KR].sum() + o[:, KR + 1].sum())
        s_x += float(o[:, KR + 2].sum())
    s_gf /= SA * SA
    s_x /= SX * SX
    gr /= SB
    s_gr = float((gr * gr).sum())
    loss = (s_gf - 2.0 * s_x + s_gr) / (2.0 * M_PAIRS)
    return np.float32(loss)


# revision 27
# speedup vs baseline: 1.0853x; 1.0439x over previous
"""Trainium2 Bass kernel for nn_ContrastiveCosineLoss.

loss = mean_{i<j} (cos(f_i,f_j) - cos(r_i,r_j))^2 over N=2048 rows.

Math: with Fn/Rn the row-normalized embeddings and
  Gf = Fn^T Fn  [1024,1024],  Gr = Rn^T Rn  [128,128],  X = Fn^T Rn  [1024,128]
  loss = (||Gf||_F^2 - 2||X||_F^2 + ||Gr||_F^2) / (2M),  M = N(N-1)/2
(diagonal diffs vanish exactly: cos(i,i)=1 on both sides).

All matmul data is fp8_e4m3 (cast on host; final loss error ~1e-3 vs the fp32
reference). fp8 enables DoubleRow matmuls and 4x less DMA than fp32.

Sharding (8 cores, SPMD single program; per-core differences only via host
packing): core c gets F8 column-rolled by c*128 (its local feature strip 0 =
global strip c) AND chunk(row)-rolled by 2c*128; R8 is chunk-rolled the same
way. Per core:
  - Gf row-strip c: lhsT = la = strip0 * (SA/nf^2), moving = raw F chunks.
  - X row-strip c: same lhsT, moving = rx = R8 * (nf/(4*nr)).
  - Gr: contraction-sharded via the first two (rolled = own) R chunks; raw
    partial Gram out, host-reduced before squaring. The chunk roll makes
    slots 0:2 a distinct 256-row slice per core covering all rows.
  - Norms nf^2/nr^2 on device, pipelined per 2-chunk DMA arrival across
    ACT/DVE (R path on GPSIMD); per-group (4 chunks) recip -> la -> Gf
    matmuls -> gg -> rx -> X matmuls: no global norm barrier anywhere.

All input DMAs ride the sync HWDGE ring (descriptor-gen runs on the issuing
engine's sequencer, so putting them on scalar/gpsimd stalls ACT compute or
faults SWDGE): front-half F in 2-chunk (256 KB) grain so squares start as
each pair lands, rap interleaved third, back-half F as two 512 KB
transfers so completion receipts pipeline.
F squares split ACT 9 / DVE 7 (GPSIMD's tensor_tensor is ~2.5x slower and
its reduce would tax DVE; it does the R squares, rx, and lb instead).
A run of garbage matmuls keeps the PE busy from t=0 toward the first real
matmul to ramp the p-state clock.

fp8 operands carry power-of-2 compensation scales (SA/SX/SB) to sit in
e4m3's normal range; the host divides them back out.
"""

import numpy as np
import ml_dtypes

N_ROWS = 2048
KF = 1024
KR = 128
P = 128
NCH = N_ROWS // P          # 16 contraction chunks of 128 rows
GRP = 4                    # chunks per scale/matmul group
NG = NCH // GRP
M_PAIRS = N_ROWS * (N_ROWS - 1) // 2
EPS2 = 1e-16               # max(norm,1e-8)^2 clamp, applied to norm^2
SA = 1024.0                # la = F8 * (SA/nf^2)
SX = 256.0                 # net X element scale: (SA/nf^2)*(nf/(4 nr)) = SX/(nf nr)
SB = 128.0                 # lb = R8 * (SB/nr^2)

TRACE = False              # test.py flips this (needs the axon NTFF shim)
LAST_EXEC_NS = None

_CACHED = {}

F8NP = ml_dtypes.float8_e4m3

def _build():
    import concourse.bacc as bacc
    import concourse.mybir as mybir
    from concourse.tile import TileContext
    from concourse.alu_op_type import AluOpType

    F32 = mybir.dt.float32
    F8 = mybir.dt.float8e4
    BF16 = mybir.dt.bfloat16
    ACTF = mybir.ActivationFunctionType
    AX = mybir.AxisListType
    DR = mybir.MatmulPerfMode.DoubleRow

    nc = bacc.Bacc("TRN2", num_devices=8)
    fmv = nc.dram_tensor("fmv", [P, NCH * KF], F8, kind="ExternalInput")
    rap = nc.dram_tensor("rap", [P, NCH * KR], F8, kind="ExternalInput")
    out_d = nc.dram_tensor("out_d", [P, KR + 3], F32, kind="ExternalOutput")

    with TileContext(nc) as tc:
        with (
            tc.tile_pool(name="big_p", bufs=1) as big_p,
            tc.tile_pool(name="scr_p", bufs=2) as scr_p,
            tc.tile_pool(name="nrm_p", bufs=1) as nrm_p,
            tc.tile_pool(name="psum", bufs=1, space="PSUM") as psum_p,
        ):
            f_all = big_p.tile([P, NCH, KF], F8)
            ra_all = big_p.tile([P, NCH, KR], F8)
            rx_all = big_p.tile([P, NCH, KR], F8)
            la_all = big_p.tile([P, NCH, P], F8, tag="la")

            # --- input DMAs: all on the sync ring, so descriptor-gen
            # stays off the compute engines' sequencers (on scalar it
            # blocks ACT dispatch; gpsimd SWDGE faulted). Front-half F in
            # 2-chunk grain for pipelined arrival, rap third, back half
            # as one 1MB transfer.
            def fpair(t):
                nc.sync.dma_start(
                    f_all[:, 2 * t:2 * t + 2, :],
                    fmv[:, 2 * t * KF:(2 * t + 2) * KF].rearrange(
                        "p (k j) -> p k j", j=KF),
                )

            fpair(0)
            fpair(1)
            nc.sync.dma_start(
                ra_all[:], rap[:].rearrange("p (k j) -> p k j", j=KR)
            )
            fpair(2)
            fpair(3)
            nc.sync.dma_start(
                f_all[:, 8:12, :],
                fmv[:, 8 * KF:12 * KF].rearrange("p (k j) -> p k j", j=KF),
            )
            nc.sync.dma_start(
                f_all[:, 12:16, :],
                fmv[:, 12 * KF:].rearrange("p (k j) -> p k j", j=KF),
            )

            # --- PE p-state warmup: keep PE busy until the first real matmul
            wsrc = nrm_p.tile([P, 2, 512], F8)
            nc.gpsimd.memset(wsrc[:], 1)
            psA0 = psum_p.tile([P, 512], F32, tag="a0", name="psA0")
            psA1 = psum_p.tile([P, 512], F32, tag="a1", name="psA1")
            psX = psum_p.tile([P, KR], F32, tag="x", name="psX")
            psB = psum_p.tile([P, KR], F32, tag="b", name="psB")
            psW = psum_p.tile([P, 512], F32, tag="w", name="psW")
            for i in range(26):
                nc.tensor.matmul(
                    psW[:], lhsT=wsrc[:, :, 0:P], rhs=wsrc[:],
                    start=True, stop=True, perf_mode=DR,
                )

            # --- ACT table warmup: Sqrt first -> sqrt_and_others set, which
            # also contains Square, so ideally a single table load.
            warm = nrm_p.tile([P, 1], F32)
            nc.vector.memset(warm[:], 1.0)
            warm2 = nrm_p.tile([P, 1], F32)
            nc.scalar.activation(warm2[:], warm[:], ACTF.Sqrt)
            nc.scalar.activation(warm2[:], warm[:], ACTF.Square)

            nf2 = nrm_p.tile([P, NCH], F32, tag="nf2")
            nr2 = nrm_p.tile([P, NCH], F32, tag="nr2")
            af_all = nrm_p.tile([P, NCH], F32, tag="af")
            rr_all = nrm_p.tile([P, NCH], F32, tag="rr")
            vv_all = nrm_p.tile([P, NCH], F32, tag="vv")
            gg_all = nrm_p.tile([P, NCH], F32, tag="gg")
            rsq = nrm_p.tile([P, NCH, KR], BF16, tag="rsq")
            out_sb = nrm_p.tile([P, KR + 3], F32, tag="osb")

            ISA = float(1.0 / SA)
            RSA = float(1.0 / np.sqrt(SA))

            def sq_act(k):
                s = scr_p.tile([P, KF], BF16, tag="sA", name=f"sA{k}")
                nc.scalar.activation(
                    s[:], f_all[:, k, :], ACTF.Square, scale=RSA,
                    accum_out=nf2[:, k:k + 1],
                )

            def sq_dve(k):
                s = scr_p.tile([P, KF], BF16, tag="sD", name=f"sD{k}")
                nc.vector.scalar_tensor_tensor(
                    s[:], f_all[:, k, :], ISA, f_all[:, k, :],
                    AluOpType.mult, AluOpType.mult,
                    accum_out=nf2[:, k:k + 1],
                )

            # R squares: two 1024-wide GPSIMD passes; segmented DVE reduces.
            def r_square(half):
                sl = slice(8 * half, 8 * (half + 1))
                nc.gpsimd.tensor_tensor(
                    rsq[:, sl, :].rearrange("p k j -> p (k j)"),
                    ra_all[:, sl, :].rearrange("p k j -> p (k j)"),
                    ra_all[:, sl, :].rearrange("p k j -> p (k j)"),
                    AluOpType.mult,
                )

            def r_reduce(half):
                sl = slice(8 * half, 8 * (half + 1))
                nc.vector.reduce_sum(nr2[:, sl], rsq[:, sl, :], axis=AX.X)
                tr = nrm_p.tile([P, 8], F32, name=f"tr{half}")
                nc.vector.tensor_scalar_max(tr[:], nr2[:, sl], EPS2)
                nc.vector.reciprocal(rr_all[:, sl], tr[:])

            def recip_la(g):
                sl = slice(GRP * g, GRP * (g + 1))
                nc.vector.reciprocal(af_all[:, sl], nf2[:, sl])
                nc.vector.tensor_tensor(
                    la_all[:, sl, :], f_all[:, sl, 0:P],
                    af_all[:, sl, None].broadcast_to([P, GRP, P]),
                    AluOpType.mult,
                )
                nc.vector.tensor_tensor(
                    vv_all[:, sl], nf2[:, sl], rr_all[:, sl], AluOpType.mult
                )

            def gf_mms(g):
                for t in (2 * g, 2 * g + 1):
                    st = dict(start=(t == 0), stop=(t == NCH // 2 - 1))
                    ksl = slice(2 * t, 2 * t + 2)
                    nc.tensor.matmul(
                        psA0[:], lhsT=la_all[:, ksl, :],
                        rhs=f_all[:, ksl, 0:512], perf_mode=DR, **st
                    )
                    nc.tensor.matmul(
                        psA1[:], lhsT=la_all[:, ksl, :],
                        rhs=f_all[:, ksl, 512:KF], perf_mode=DR, **st
                    )

            def gg_act(g):
                sl = slice(GRP * g, GRP * (g + 1))
                # gg = sqrt(vv * SA/16) = nf/(4 nr); scale applies inside.
                nc.scalar.activation(
                    gg_all[:, sl], vv_all[:, sl], ACTF.Sqrt, scale=SA / 16.0
                )

            def rx_gps(g):
                sl = slice(GRP * g, GRP * (g + 1))
                nc.gpsimd.tensor_tensor(
                    rx_all[:, sl, :], ra_all[:, sl, :],
                    gg_all[:, sl, None].broadcast_to([P, GRP, KR]),
                    AluOpType.mult,
                )

            def x_mms(g):
                for t in (2 * g, 2 * g + 1):
                    st = dict(start=(t == 0), stop=(t == NCH // 2 - 1))
                    ksl = slice(2 * t, 2 * t + 2)
                    nc.tensor.matmul(
                        psX[:], lhsT=la_all[:, ksl, :],
                        rhs=rx_all[:, ksl, :], perf_mode=DR, **st
                    )

            # --- emission: squares in arrival order; ACT 9 / DVE 7;
            # GPSIMD: R squares, rx, lb.
            r_square(0)                      # GPS (rap lands third)
            sq_act(0)
            sq_dve(1)
            sq_act(2)
            sq_dve(3)
            r_reduce(0)                      # DVE

            # Gr: lb from the first two (own) R chunks; raw rhs.
            nb = nrm_p.tile([P, 2], F32, tag="nb")
            nc.vector.tensor_scalar(
                nb[:], nr2[:, 0:2], EPS2, float(1.0 / SB),
                AluOpType.max, AluOpType.mult,
            )
            bb = nrm_p.tile([P, 2], F32, tag="bb")
            nc.vector.reciprocal(bb[:], nb[:])
            lb = nrm_p.tile([P, 2, KR], F8, tag="lb")
            nc.gpsimd.tensor_tensor(
                lb[:], ra_all[:, 0:2, :],
                bb[:, :, None].broadcast_to([P, 2, KR]), AluOpType.mult,
            )

            r_square(1)                      # GPS
            sq_act(4)
            sq_dve(5)
            recip_la(0)
            gf_mms(0)
            nc.tensor.matmul(
                psB[:], lhsT=lb[:], rhs=ra_all[:, 0:2, :],
                start=True, stop=True, perf_mode=DR,
            )
            sq_act(6)
            sq_dve(7)
            r_reduce(1)                      # DVE
            gg_act(0)
            rx_gps(0)
            sq_act(8)
            sq_dve(9)
            recip_la(1)
            gf_mms(1)
            x_mms(0)
            gg_act(1)
            rx_gps(1)
            sq_act(10)
            sq_dve(11)
            recip_la(2)
            gf_mms(2)
            x_mms(1)
            gg_act(2)
            rx_gps(2)
            sq_act(12)
            sq_dve(13)
            x_mms(2)
            sq_act(14)
            sq_act(15)
            recip_la(3)
            # G3 matmuls: psA0's pair-run first so it finalizes two MMs
            # early and its Frobenius square overlaps psA1's matmuls.
            for t in (6, 7):
                nc.tensor.matmul(
                    psA0[:], lhsT=la_all[:, 2 * t:2 * t + 2, :],
                    rhs=f_all[:, 2 * t:2 * t + 2, 0:512], perf_mode=DR,
                    start=False, stop=(t == 7),
                )
            sE0 = scr_p.tile([P, 512], F32, tag="sE0")
            nc.scalar.activation(
                sE0[:], psA0[0:P, :], ACTF.Square,
                accum_out=out_sb[:, KR:KR + 1],
            )
            for t in (6, 7):
                nc.tensor.matmul(
                    psA1[:], lhsT=la_all[:, 2 * t:2 * t + 2, :],
                    rhs=f_all[:, 2 * t:2 * t + 2, 512:KF], perf_mode=DR,
                    start=False, stop=(t == 7),
                )
            gg_act(3)
            # rx3 on DVE: shorter latency than GPSIMD on the final chain
            nc.vector.tensor_tensor(
                rx_all[:, 12:16, :], ra_all[:, 12:16, :],
                gg_all[:, 12:16, None].broadcast_to([P, GRP, KR]),
                AluOpType.mult,
            )
            x_mms(3)

            # --- epilogue: Frobenius partials + Gr partial -> one DMA ---
            sE1 = scr_p.tile([P, 512], F32, tag="sE1")
            nc.scalar.activation(
                sE1[:], psA1[0:P, :], ACTF.Square,
                accum_out=out_sb[:, KR + 1:KR + 2],
            )
            nc.vector.tensor_copy(out_sb[:, 0:KR], psB[0:P, :])
            sX1 = scr_p.tile([P, KR], F32, tag="sX1")
            nc.vector.tensor_copy(sX1[:], psX[0:P, :])
            sX2 = scr_p.tile([P, KR], F32, tag="sX2")
            nc.vector.scalar_tensor_tensor(
                sX2[:], sX1[:], 1.0, sX1[:],
                AluOpType.mult, AluOpType.mult,
                accum_out=out_sb[:, KR + 2:KR + 3],
            )
            nc.sync.dma_start(out_d[:], out_sb[:])

    nc.finalize()
    return nc


def _pack_rolled(a, nch, roll):
    # [nch*128, w] row-chunked -> chunk-rolled [128, nch*w] SBUF-native
    w = a.shape[1]
    ch = a.reshape(nch, P, w)
    if roll:
        ch = np.roll(ch, -roll, axis=0)
    return np.ascontiguousarray(ch.transpose(1, 0, 2).reshape(P, nch * w))


def kernel(reduced_embeddings: np.ndarray, full_embeddings: np.ndarray) -> np.ndarray:
    global LAST_EXEC_NS
    from concourse.bass_utils import run_bass_kernel_spmd

    F8 = full_embeddings.astype(F8NP)
    R8 = reduced_embeddings.astype(F8NP)

    if "nc" not in _CACHED:
        _CACHED["nc"] = _build()
    nc = _CACHED["nc"]

    in_maps = []
    for c in range(8):
        fa = np.roll(F8, -(c * P), axis=1)
        in_maps.append({
            "fmv": _pack_rolled(fa, NCH, 2 * c),
            "rap": _pack_rolled(R8, NCH, 2 * c),
        })

    kw = {}
    if TRACE:
        kw = dict(trace=True, trace_cores=[0])
    res = run_bass_kernel_spmd(nc, in_maps, core_ids=list(range(8)), **kw)
    LAST_EXEC_NS = res.exec_time_ns

    s_gf = 0.0
    s_x = 0.0
    gr = np.zeros((P, KR), dtype=np.float64)
    for c in range(8):
        o = res.results[c]["out_d"].astype(np.float64)
        gr += o[:, 0:KR]
        s_gf += float(o[:, KR].sum() + o[:, KR + 1].sum())
        s_x += float(o[:, KR + 2].sum())
    s_gf /= SA * SA
    s_x /= SX * SX
    gr /= SB
    s_gr = float((gr * gr).sum())
    loss = (s_gf - 2.0 * s_x + s_gr) / (2.0 * M_PAIRS)
    return np.float32(loss)
